# revision 25
# baseline (speedup 1.0000x reference)
"""Trainium2 Bass kernel for nn_Brain (gnn_message_passing, N=20000, E=20M, B=4, S=8).

Math (faithful to the reference):
    a_0 = zeros(N); a_0[:1000] = x0
    total_u[j] = c0[j] + sum_{d=1..u-1} sum_{e in E_d} w_e * a_{u-d}[from_e]   (to_e = j)
    c0[j]      = sum_{delay-0 edges} w_e * a_0[from_e]     (constant across steps)
    a_u = tanh(total_u), u = 1..8;  output = a_8[-1000:]   (delays >= 8 never fire)

v4: fp8 DoubleRow matmuls (the v2 PE bottleneck was ~2.7ms at 1 col/cycle;
DoubleRow contracts a 256-row superblock per pass).  Both operands must be
fp8: planes stay fp8e4m3 (x WSCALE), snapshots are stored fp8e4m3 x SSCALE.
w0/a0 move to bf16 (w0 host-scaled by WSCALE*SSCALE); tanh scale =
1/(WSCALE*SSCALE).

DoubleRow ISA restrictions (walrus s3_lw_dual_fp8_restrictions):
  - matmul PSUM dst must start at partition 0 (no column tiling), and
  - stationary free-dim steps beyond the innermost must be 16B-aligned.
So the v2 rotating 32-partition PSUM regions are gone.  Instead each
interval k shares ONE accumulation region at partitions 0..4*(S-k):
row-block (u-k-1)*b holds the merged contribution to step u from all of
this interval's reads.  Row alignment comes from the stationary: the fp8
snapshot table has, per (superblock, ktile), a 20-zero prefix + 28 snapshot
slots (sp=48, so the ktile step 48 is 16B-aligned), and a read with
snapshot range slo..shi and row offset o = slo+d-k-1 loads sp[20+(slo-1-o)*4
: 20+shi*4], contracting zeros into rows below its targets.  At interval
end the DVE stages rows 0..nu_k to a rotating bf16 SBUF buffer and releases
PSUM (PE stall ~2us/interval); totals and the per-(k,u) future shifts
(scalar-queue partition-shift DMAs -> scratch -> acc_u adds) read the
staged copy.  c0 lives at PSUM partitions 32-35 (plain bf16 matmul, legal).

Read schedule (gate = highest snapshot used; gate<k reads run before the
step-k AllGather's transposes and absorb the collective latency):
  k=1:                  d1[s1]
  k=2: d7[s1](g1,restr) d2[s1..2] d1[s2]
  k=3: d6[s1..2](g2)    d3[s1..3] d1[s3]
  k=4: d5[s1..3](g3)    d4[s1..4] d2[s3..4] d1[s4]
  k=5:                  d1[s5]
  k=6: d3[s4..5](g5)    d2[s5..6] d1[s6]
  k=7:                  d1[s7](restr)
Within an interval the first writer of each PSUM bank must cover all rows
consumed from that bank (start=True coverage; validated statically).
"""
import sys
sys.path.insert(0, '/opt/trn_rl_repo')
import numpy as np
import ml_dtypes

NC_COUNT = 8
WSCALE = 64.0
SSCALE = 8.0
ZPFX = 20                                   # zero-prefix slots in sp dim
SPW = 48                                    # sp dim width (20 zeros + 28)

FULL_CFG = dict(n=20000, e_in=1000, b=4, steps=8, nbank=5, chunk_fb=8, nbuf=4,
                nrot=2, w1f=6)


def derive(cfg):
    c = dict(cfg)
    n, b, s = c['n'], c['b'], c['steps']
    jp = n // NC_COUNT                      # to-neurons per core (2500)
    jpad = ((jp + 127) // 128) * 128        # 2560
    c.update(
        jp=jp, jpad=jpad,
        lfb=jpad // 128,                    # local from-blocks per core (20)
        nfb=NC_COUNT * (jpad // 128),       # global from-blocks (160)
        fpad=NC_COUNT * jpad,               # padded from-rows (20480)
        nfb0=(c['e_in'] + 127) // 128,      # delay-0 from-blocks (8)
        bank_j=jp // c['nbank'],            # 500
        ntr=jpad // 128,                    # post-gather transpose chunks (20)
    )
    assert jp % c['nbank'] == 0 and c['bank_j'] <= 512
    assert c['chunk_fb'] % 2 == 0 and c['nfb'] % 2 == 0
    assert (s - 1) * c['b'] + ZPFX <= SPW and SPW % 16 == 0
    # ring chunks for d1 start at w1f; chunks_of handles a (necessarily even)
    # remainder chunk, which DoubleRow pairing requires
    assert c['w1f'] % 2 == 0 and 0 <= c['w1f'] < c['nfb']
    return c


def _mybir():
    import concourse.mybir as mybir
    return mybir


def make_reads(S, b):
    """Read schedule: list of dicts in PE issue order.

    gate = highest snapshot the read uses (PE gates on its copies only).
    o    = row-block offset: snapshot s lands at PSUM rows (s+d-k-1)*b.
    span = o*b + b*ns rows written (zero-prefix rows below the targets).
    """
    reads = [dict(d=0, slo=0, shi=0, itv=0, must=True, restr=False, gate=0,
                  o=0, span=b)]
    TB = {
        1: [(1, 1, 1)],
        2: [(7, 1, 1), (2, 1, 2), (1, 2, 2)],
        3: [(6, 1, 2), (3, 1, 3), (1, 3, 3)],
        4: [(5, 1, 3), (4, 1, 4), (2, 3, 4), (1, 4, 4)],
        5: [(1, 5, 5)],
        6: [(3, 4, 5), (2, 5, 6), (1, 6, 6)],
        7: [(1, 7, 7)],
    }
    for k in range(1, S):
        for d, slo, shi in TB[k]:
            assert shi <= k and slo + d >= k + 1 and shi + d <= S
            o = slo + d - k - 1
            reads.append(dict(d=d, slo=slo, shi=shi, itv=k, gate=shi,
                              must=(slo + d == k + 1), o=o,
                              span=(o + shi - slo + 1) * b,
                              restr=(slo + d == S and shi + d == S)))
    # early (gate<k) reads must precede gated ones (PE emission order is the
    # list order and the threads assume it)
    for k in range(1, S):
        kr = [r for r in reads if r['itv'] == k]
        assert kr == sorted(kr, key=lambda r: r['gate'] >= k)
        # spans must be non-increasing so each bank's first writer covers
        # every row consumed from that bank this interval
        spans = [r['span'] for r in kr]
        assert spans == sorted(spans, reverse=True)
    # coverage: every (d, u) pair exactly once, on time
    seen = set()
    for r in reads[1:]:
        for s in range(r['slo'], r['shi'] + 1):
            key = (r['d'], s + r['d'])
            assert key not in seen and s + r['d'] <= S
            assert s + r['d'] >= r['itv'] + 1
            seen.add(key)
    assert seen == {(d, u) for d in range(1, S) for u in range(d + 1, S + 1)}
    return reads


def interval_info(reads, S, b, nbank, nbk_r):
    """Per-interval: nu (rows staged), consumed (u -> (row0, restr_src)),
    and per-read per-bank start/stop flags."""
    info = {}
    for k in range(1, S):
        kr = [(ri, r) for ri, r in enumerate(reads) if r['itv'] == k]
        nu = max(r['span'] for _, r in kr)
        consumed = {}
        for _, r in kr:
            for i in range(r['shi'] - r['slo'] + 1):
                u = r['slo'] + r['d'] + i
                ro = (r['o'] + i) * b
                prev = consumed.get(u)
                if prev is not None:
                    assert prev == (ro, r['restr']), 'mixed src windows'
                consumed[u] = (ro, r['restr'])
        # per-bank first/last writer -> start/stop flags per read
        first_w, last_w = {}, {}
        for ri, r in kr:
            banks = range(nbk_r) if r['restr'] else range(nbank)
            for bi in banks:
                first_w.setdefault(bi, ri)
                last_w[bi] = ri
        starts, stops = {}, {}
        for ri, r in kr:
            banks = range(nbk_r) if r['restr'] else range(nbank)
            starts[ri] = {bi: (first_w[bi] == ri) for bi in banks}
            stops[ri] = {bi: (last_w[bi] == ri) for bi in banks}
            # coverage check: first writer of each bank spans all consumed
            # rows of that bank this interval
            for bi in banks:
                if first_w[bi] != ri:
                    continue
                for u, (ro, rsrc) in consumed.items():
                    src_banks = range(nbk_r) if rsrc else range(nbank)
                    if bi in src_banks or u == k + 1:
                        assert ro + b <= r['span'], (k, u, bi)
        info[k] = dict(nu=nu, consumed=consumed, starts=starts, stops=stops)
    return info


# --------------------------------------------------------------------------
# Bass program
# --------------------------------------------------------------------------
def build_bass(cfg, reps=1):
    from concourse import bass
    mybir = _mybir()
    c = derive(cfg)
    n, b, S = c['n'], c['b'], c['steps']
    jp, jpad, lfb, nfb, nfb0 = c['jp'], c['jpad'], c['lfb'], c['nfb'], c['nfb0']
    nbank, bank_j, chunk_fb = c['nbank'], c['bank_j'], c['chunk_fb']
    NBUF = c['nbuf']
    NROT = c['nrot']
    ntr = c['ntr']
    NSNAP = S - 1
    TGRP = 16
    ngrp = (ntr + TGRP - 1) // TGRP
    e_in = c['e_in']
    jr = jp - e_in                          # restricted col start (1500)
    rbank0 = jr // bank_j                   # first bank of restricted cols (3)
    nbk_r = e_in // bank_j                  # banks in restricted reads (2)
    assert rbank0 * bank_j == jr

    reads = make_reads(S, b)
    NREADS = len(reads)                     # 17
    iinfo = interval_info(reads, S, b, nbank, nbk_r)

    rhs_elems = chunk_fb * jp               # ring buf bytes per partition (20000)
    w0_ch = chunk_fb // 2                   # bf16 fb per chunk (4)
    W1F = c['w1f']                          # d1 from-blocks resident in SBUF

    def chunks_of(lo, total, ch):
        out, x = [], lo
        while x < total:
            out.append((x, min(ch, total - x)))
            x += ch
        return out

    chunk_list = []
    cum_end = []                            # per-read end chunk index (per rep)
    for ri, r in enumerate(reads):
        if r['d'] == 0:
            ch_list = chunks_of(0, nfb0, w0_ch)
        elif r['d'] == 1:
            # first W1F blocks come from the resident SBUF copy, not the ring
            ch_list = chunks_of(W1F, nfb, chunk_fb)
        else:
            ch_list = chunks_of(0, nfb, chunk_fb)
        for (f0, ch) in ch_list:
            chunk_list.append((ri, f0, ch))
        cum_end.append(len(chunk_list))
    NCHUNK = len(chunk_list)

    # shift plan: per (k, u>k+1): one partition-shift DMA + one acc add.
    # acc_us: accumulator targets.
    shift_plan = []                         # (k, u, ro, restr_src)
    for k in range(1, S):
        for u in sorted(iinfo[k]['consumed']):
            if u == k + 1:
                continue
            ro, rsrc = iinfo[k]['consumed'][u]
            shift_plan.append((k, u, ro, rsrc))
    n_shifts = len(shift_plan)
    acc_us = sorted({u for (_, u, _, _) in shift_plan})
    # per-u first-shift index (to pick copy vs add into acc) per rep
    first_shift_of_u = {}
    for jx, (k, u, _, _) in enumerate(shift_plan):
        first_shift_of_u.setdefault(u, jx)
    # tanh gating: tot_sem incs are c0-copy (u=1) then one per interval total
    # -> tanh(u) waits tot_sem >= rep*S + u.

    nc = bass.Bass(target_bir_lowering=False)

    planes = [nc.declare_dram_parameter(f'w{d}', [128, nfb * jp], mybir.dt.uint8,
                                        isOutput=False) for d in range(1, S)]
    w0_t = nc.declare_dram_parameter('w0', [128, nfb0 * jp], mybir.dt.bfloat16,
                                     isOutput=False)
    a0_t = nc.declare_dram_parameter('a0', [128, nfb0 * b], mybir.dt.bfloat16,
                                     isOutput=False)
    id_t = nc.declare_dram_parameter('ident', [32, 32], mybir.dt.bfloat16,
                                     isOutput=False)
    out_t = nc.declare_dram_parameter('a8', [b, e_in], mybir.dt.float32,
                                      isOutput=True)
    ag_in = nc.dram_tensor('ag_in', [b, jpad], mybir.dt.bfloat16)
    ag_out = nc.dram_tensor('ag_out', [NC_COUNT * b, jpad], mybir.dt.bfloat16)

    from contextlib import ExitStack
    with ExitStack() as _es:
        init_sem = _es.enter_context(nc.semaphore('init_sem'))
        w1c_sem = _es.enter_context(nc.semaphore('w1c_sem'))
        pln_sems = [_es.enter_context(nc.semaphore(f'pln{i}')) for i in range(NBUF)]
        free_sem = _es.enter_context(nc.semaphore('free_sem'))
        drain_sem = _es.enter_context(nc.semaphore('drain_sem'))
        tot_sem = _es.enter_context(nc.semaphore('tot_sem'))
        act_sem = _es.enter_context(nc.semaphore('act_sem'))
        agd_sem = _es.enter_context(nc.semaphore('agd_sem'))
        cc_sem = _es.enter_context(nc.semaphore('cc_sem'))
        tr_sem = _es.enter_context(nc.semaphore('tr_sem'))
        cp_sem = _es.enter_context(nc.semaphore('cp_sem'))
        ms_sem = _es.enter_context(nc.semaphore('ms_sem'))
        fin_sem = _es.enter_context(nc.semaphore('fin_sem'))
        shf_sem = _es.enter_context(nc.semaphore('shf_sem'))
        scr_sem = _es.enter_context(nc.semaphore('scr_sem'))
        sb_rhs = [_es.enter_context(nc.sbuf_tensor(f'sb_rhs{i}', [128, rhs_elems], mybir.dt.uint8))
                  for i in range(NBUF)]
        sb_w1c = (_es.enter_context(nc.sbuf_tensor('sb_w1c', [128, W1F * jp], mybir.dt.uint8))
                  if W1F else None)
        sb_snap = _es.enter_context(nc.sbuf_tensor('sb_snap', [128, nfb * SPW], mybir.dt.float8e4))
        sb_a0 = _es.enter_context(nc.sbuf_tensor('sb_a0', [128, nfb0 * b], mybir.dt.bfloat16))
        sb_id = _es.enter_context(nc.sbuf_tensor('sb_id', [32, 32], mybir.dt.bfloat16))
        sb_c0 = _es.enter_context(nc.sbuf_tensor('sb_c0', [b, jp], mybir.dt.float32))
        sb_tot = _es.enter_context(nc.sbuf_tensor('sb_tot', [b, jp], mybir.dt.float32))
        sb_tmp = _es.enter_context(nc.sbuf_tensor('sb_tmp', [b, jp], mybir.dt.float32))
        sb_tot8 = _es.enter_context(nc.sbuf_tensor('sb_tot8', [b, e_in], mybir.dt.float32))
        sb_a = _es.enter_context(nc.sbuf_tensor('sb_a', [b, jpad], mybir.dt.bfloat16))
        sb_a32 = _es.enter_context(nc.sbuf_tensor('sb_a32', [b, e_in], mybir.dt.float32))
        sb_ag = _es.enter_context(nc.sbuf_tensor('sb_ag', [NC_COUNT * b, jpad], mybir.dt.bfloat16))
        sb_acc = {u: _es.enter_context(nc.sbuf_tensor(
            f'sb_acc{u}', [b, e_in if u == S else jp], mybir.dt.float32))
            for u in acc_us}
        # staged PSUM rows (bf16, rotating): totals + shifts read from here
        sb_stg = _es.enter_context(nc.sbuf_tensor('sb_stg', [128, NROT * jp], mybir.dt.bfloat16))
        sb_scr = _es.enter_context(nc.sbuf_tensor('sb_scr', [b, jp], mybir.dt.bfloat16))
        ps = _es.enter_context(nc.psum_tensor('ps', [128, nbank * 512], mybir.dt.float32))
        psT = _es.enter_context(nc.psum_tensor('psT', [128, 2 * TGRP * NC_COUNT * b], mybir.dt.bfloat16))

        block = _es.enter_context(nc.Block())
        AFT = mybir.ActivationFunctionType
        DR = mybir.MatmulPerfMode.DoubleRow
        psr = ps.ap().rearrange('p (k j) -> p k j', k=nbank)
        # snapshot table: [p, superblock f2, ktile t, sp] with 20-zero prefix
        snapv = sb_snap.ap().rearrange('p (f2 t sp) -> p f2 t sp',
                                       f2=nfb // 2, t=2)
        a03 = sb_a0.ap().rearrange('p (f b) -> p f b', f=nfb0)
        psT4 = psT.ap().rearrange('p (h t cb) -> p h t cb', h=2, t=TGRP)
        sb_ag3 = sb_ag.ap().rearrange('cb (k p) -> cb k p', p=128)
        c0_3 = sb_c0.ap().rearrange('p (k j) -> p k j', k=nbank)
        tot_3 = sb_tot.ap().rearrange('p (k j) -> p k j', k=nbank)
        tmp_3 = sb_tmp.ap().rearrange('p (k j) -> p k j', k=nbank)
        tot8_3 = sb_tot8.ap().rearrange('p (k j) -> p k j', k=nbk_r)
        stg4 = sb_stg.ap().rearrange('p (r k j) -> p r k j', r=NROT, k=nbank)

        def acc_3(u):
            k = nbk_r if u == S else nbank
            return sb_acc[u].ap().rearrange('p (k j) -> p k j', k=k)

        def rot_of(k_abs):
            return k_abs % NROT

        # -------------------------------------------- sync: plane DMA stream
        @block.sync
        def _(s):
            s.dma_start(out=sb_a0[:, :], in_=a0_t[:, :]).then_inc(init_sem, 16)
            s.dma_start(out=sb_id[:, :], in_=id_t[:, :]).then_inc(init_sem, 16)
            if W1F:
                # one-time fill of the resident d1-plane prefix (reused by
                # every d1 read of every rep)
                s.dma_start(out=sb_w1c[:, :],
                            in_=planes[0].ap()[:, 0:W1F * jp]).then_inc(w1c_sem, 16)
            gi = 0
            for rep in range(reps):
                for (ri, f0, ch) in chunk_list:
                    r = reads[ri]
                    if gi >= NBUF:
                        s.wait_ge(free_sem, gi - NBUF + 1)
                    buf = sb_rhs[gi % NBUF]
                    if r['d'] == 0:
                        src = w0_t.ap().rearrange('p (f j) -> p f j', f=nfb0)[
                            :, f0:f0 + ch, :]
                        dst = buf.ap().bitcast(mybir.dt.bfloat16)[
                            :, 0:ch * jp].rearrange('p (c j) -> p c j', c=ch)
                    elif r['restr']:
                        src = planes[r['d'] - 1].ap().rearrange(
                            'p (f j) -> p f j', f=nfb)[:, f0:f0 + ch, jr:jp]
                        dst = buf[:, 0:ch * e_in].rearrange('p (c j) -> p c j', c=ch)
                    else:
                        src = planes[r['d'] - 1].ap().rearrange(
                            'p (f j) -> p f j', f=nfb)[:, f0:f0 + ch, :]
                        dst = buf[:, 0:ch * jp].rearrange('p (c j) -> p c j', c=ch)
                    s.dma_start(out=dst, in_=src).then_inc(pln_sems[gi % NBUF], 16)
                    gi += 1
                s.wait_ge(act_sem, rep * S + S)
                s.dma_start(out=out_t[:, :], in_=sb_a32[:, :]).then_inc(fin_sem, 16)
            s.wait_ge(fin_sem, 16 * reps)

        # -------------------------------------------- tensor: matmuls + transposes
        @block.tensor
        def _(t):
            ci = 0
            gtr = 0

            def emit_read(ri, rep, k):
                nonlocal ci
                r = reads[ri]
                d, slo, shi, o = r['d'], r['slo'], r['shi'], r['o']
                nbk = nbk_r if r['restr'] else nbank
                jw = e_in if r['restr'] else jp
                span = r['span']
                if r['gate'] >= 1:
                    t.wait_ge(cp_sem, (rep * (S - 1) + r['gate']) * ngrp)
                starts = iinfo[k]['starts'][ri] if k >= 1 else None
                stops = iinfo[k]['stops'][ri] if k >= 1 else None
                cached = (d == 1 and W1F > 0)
                if cached:
                    # resident d1-plane prefix: matmuls straight out of SBUF
                    sp0 = ZPFX + (slo - 1 - o) * b
                    rhs4c = sb_w1c.ap().bitcast(mybir.dt.float8e4).rearrange(
                        'p (c2 t j) -> p c2 t j', c2=W1F // 2, t=2)
                    jb = jr if r['restr'] else 0
                    t.wait_ge(w1c_sem, 16)
                    if r['gate'] >= 1:
                        pass  # cp wait already emitted above
                    for cc2 in range(W1F // 2):
                        lhsT = snapv[:, cc2, :, sp0:sp0 + span]
                        for bi in range(nbk):
                            mm = t.matmul(
                                psr[0:span, bi, 0:bank_j],
                                lhsT,
                                rhs4c[:, cc2, :,
                                      jb + bi * bank_j:jb + (bi + 1) * bank_j],
                                start=(starts[bi] and cc2 == 0),
                                stop=False,
                                perf_mode=DR,
                                skip_group_check=True)
                first_c, last_c = (cum_end[ri - 1] if ri else 0), cum_end[ri]
                for cj in range(first_c, last_c):
                    _, f0, ch = chunk_list[cj]
                    if ci == 0:
                        t.wait_ge(init_sem, 32)
                    t.wait_ge(pln_sems[ci % NBUF], 16 * (ci // NBUF + 1))
                    buf = sb_rhs[ci % NBUF]
                    if d == 0:
                        rhs3 = buf.ap().bitcast(mybir.dt.bfloat16)[
                            :, 0:ch * jw].rearrange('p (c j) -> p c j', c=ch)
                        for cc in range(ch):
                            lhsT = a03[:, f0 + cc, :]
                            for bi in range(nbk):
                                mm = t.matmul(
                                    psr[32:32 + b, bi, 0:bank_j],
                                    lhsT,
                                    rhs3[:, cc, bi * bank_j:(bi + 1) * bank_j],
                                    start=(cj == first_c and cc == 0),
                                    stop=(cj == last_c - 1 and cc == ch - 1),
                                    skip_group_check=True)
                    else:
                        # DoubleRow: [p, 2, jw] moving vs [p, 2, sp-slice] fp8
                        rhs4 = buf.ap().bitcast(mybir.dt.float8e4)[
                            :, 0:ch * jw].rearrange('p (c2 t j) -> p c2 t j',
                                                    c2=ch // 2, t=2)
                        sp0 = ZPFX + (slo - 1 - o) * b
                        for cc2 in range(ch // 2):
                            f2 = (f0 + 2 * cc2) // 2
                            lhsT = snapv[:, f2, :, sp0:sp0 + span]
                            for bi in range(nbk):
                                mm = t.matmul(
                                    psr[0:span, bi, 0:bank_j],
                                    lhsT,
                                    rhs4[:, cc2, :, bi * bank_j:(bi + 1) * bank_j],
                                    start=(starts[bi] and cj == first_c
                                           and cc2 == 0 and not cached),
                                    stop=(stops[bi] and cj == last_c - 1
                                          and cc2 == ch // 2 - 1),
                                    perf_mode=DR,
                                    skip_group_check=True)
                    mm.then_inc(free_sem, 1)
                    ci += 1

            for rep in range(reps):
                for k in range(0, S):
                    if k >= 1:
                        # region reuse: stages of intervals 1..k-1 (this rep)
                        # and all of the previous rep must be done
                        t.wait_ge(drain_sem, rep * (S - 1) + k - 1)
                    else:
                        if rep > 0:
                            # c0 PSUM region (parts 32+) reuse: previous rep's
                            # c0 copy must be done
                            t.wait_ge(tot_sem, (rep - 1) * S + 1)
                    for ri, r in enumerate(reads):
                        if r['itv'] == k and r['gate'] < k:
                            emit_read(ri, rep, k)
                    if 1 <= k <= S - 1:
                        gs = rep * (S - 1) + k
                        t.wait_ge(agd_sem, 32 * gs)
                        for g in range(ngrp):
                            if gtr >= 2:
                                t.wait_ge(cp_sem, gtr - 1)
                            half = gtr % 2
                            k0 = g * TGRP
                            kcnt = min(TGRP, ntr - k0)
                            for kk in range(kcnt):
                                mm = t.transpose(psT4[:, half, kk, :],
                                                 sb_ag3[:, k0 + kk, :], sb_id[:, :])
                            mm.then_inc(tr_sem, 1)
                            gtr += 1
                    for ri, r in enumerate(reads):
                        if r['itv'] == k and r['gate'] >= k:
                            emit_read(ri, rep, k)

        # -------------------------------------------- scalar: tanh + shift DMAs
        @block.scalar
        def _(a):
            a.wait_ge(ms_sem, 2)
            jshift = 0
            for rep in range(reps):
                for u in range(1, S + 1):
                    a.wait_ge(tot_sem, rep * S + u)
                    if u == 1:
                        a.activation(sb_a[:, 0:jp], sb_c0[:, :], AFT.Tanh,
                                     scale=1.0 / (WSCALE * SSCALE)).then_inc(act_sem, 1)
                    elif u < S:
                        a.activation(sb_a[:, 0:jp], sb_tot[:, :], AFT.Tanh,
                                     scale=1.0 / (WSCALE * SSCALE)).then_inc(act_sem, 1)
                    else:
                        if rep > 0:
                            a.wait_ge(fin_sem, 16 * rep)
                        a.activation(sb_a32[:, :], sb_tot8[:, :], AFT.Tanh,
                                     scale=1.0 / (WSCALE * SSCALE)).then_inc(act_sem, 1)
                    # interval k=u-1's future shifts: stg -> scratch
                    if u <= S - 1:
                        k = u - 1
                        for (kk2, uu, ro, rsrc) in shift_plan:
                            if kk2 != k:
                                continue
                            # stage(k) done is implied by tot_sem(u) above
                            if jshift >= 1:
                                a.wait_ge(scr_sem, jshift)
                            nbs = nbk_r if uu == S else nbank
                            blo = 0 if (uu < S or rsrc) else rbank0
                            w = nbs * bank_j
                            rr = rot_of(rep * (S - 1) + k - 1)
                            a.dma_start(
                                out=sb_scr.ap()[0:b, 0:w],
                                in_=stg4[ro:ro + b, rr, blo:blo + nbs, :],
                            ).then_inc(shf_sem, 16)
                            jshift += 1

        # -------------------------------------------- vector: stage + totals + snap copies
        @block.vector
        def _(v):
            gcp = 0
            jadd = 0
            cls_last = {}
            v.memset(sb_a[:, :], 0.0).then_inc(ms_sem, 1)
            v.memset(sb_snap[:, :], 0.0).then_inc(ms_sem, 1)
            for rep in range(reps):

                def acc_write(u, src, kk, jx):
                    if jx == first_shift_of_u[u]:
                        return v.tensor_copy(acc_3(u)[:, 0:kk, :], src)
                    v.tensor_tensor(tmp_3[:, 0:kk, :],
                                    acc_3(u)[:, 0:kk, :], src,
                                    mybir.AluOpType.add)
                    return v.tensor_copy(acc_3(u)[:, 0:kk, :],
                                         tmp_3[:, 0:kk, :])

                # w0 -> c0 (PSUM partitions 32..32+b)
                v.wait_ge(free_sem, rep * NCHUNK + cum_end[0])
                v.tensor_copy(c0_3, psr[32:32 + b, 0:nbank, 0:bank_j]
                              ).then_inc(tot_sem, 1)

                for k in range(1, S):
                    # snapshot-k copies (scaled bf16 -> fp8)
                    for g in range(ngrp):
                        v.wait_ge(tr_sem, gcp + 1)
                        half = gcp % 2
                        k0 = g * TGRP
                        kcnt = min(TGRP, ntr - k0)
                        src = psT4[:, half, 0:kcnt, :].rearrange(
                            'p t (c b) -> p t c b', c=NC_COUNT)
                        dst = sb_snap.ap().rearrange(
                            'p (c kl sp) -> p kl c sp', c=NC_COUNT, sp=SPW)[
                            :, k0:k0 + kcnt, :,
                            ZPFX + (k - 1) * b:ZPFX + k * b]
                        v.tensor_scalar_mul(dst, src, SSCALE).then_inc(cp_sem, 1)
                        gcp += 1

                    u = k + 1
                    nu = iinfo[k]['nu']
                    last_ri = max(ri for ri, r in enumerate(reads)
                                  if r['itv'] == k)
                    v.wait_ge(free_sem, rep * NCHUNK + cum_end[last_ri])
                    # stage rows 0..nu to the rotating bf16 buffer, release PSUM
                    rr = rot_of(rep * (S - 1) + k - 1)
                    if rr in cls_last:
                        v.wait_ge(shf_sem, 16 * cls_last[rr])
                    v.tensor_copy(stg4[0:nu, rr, :, :],
                                  psr[0:nu, 0:nbank, 0:bank_j]
                                  ).then_inc(drain_sem, 1)
                    lastj = [jx for jx, (kk2, _, _, _) in enumerate(shift_plan)
                             if kk2 == k]
                    if lastj:
                        cls_last[rr] = rep * n_shifts + lastj[-1] + 1
                    # total_u = c0 + stg rows 0..b + acc_u
                    direct_restr = reads[last_ri]['restr'] and u == S
                    addends = [('c0', None), ('stg', rr)]
                    # every shift (k', u) has k' <= u-2 = k-1, so acc_u is
                    # complete by now whenever it exists at all
                    if u in sb_acc:
                        addends.append(('acc', u))
                    A = len(addends)
                    if u < S:
                        c0v, totv, tmpv = c0_3, tot_3, tmp_3
                        stgv = stg4[0:b, rr, 0:nbank, :]
                    else:
                        c0v = c0_3[:, rbank0:rbank0 + nbk_r, :]
                        totv = tot8_3
                        tmpv = tmp_3[:, 0:nbk_r, :]
                        blo = 0 if direct_restr else rbank0
                        stgv = stg4[0:b, rr, blo:blo + nbk_r, :]

                    def addend_ap(spec):
                        kind, x = spec
                        if kind == 'c0':
                            return c0v
                        if kind == 'stg':
                            return stgv
                        kk = nbk_r if u == S else nbank
                        return acc_3(u)[:, 0:kk, :]

                    cur = addend_ap(addends[0])
                    op = None
                    for j in range(A - 1):
                        target = totv if (A - 2 - j) % 2 == 0 else tmpv
                        op = v.tensor_tensor(target, cur, addend_ap(addends[j + 1]),
                                             mybir.AluOpType.add)
                        cur = target
                    op.then_inc(tot_sem, 1)
                    # scratch adds for interval k's shifts
                    for jx, (kk2, uu, ro, rsrc) in enumerate(shift_plan):
                        if kk2 != k:
                            continue
                        v.wait_ge(shf_sem, 16 * (jadd + 1))
                        kk = nbk_r if uu == S else nbank
                        w = e_in if uu == S else jp
                        scr = sb_scr.ap()[0:b, 0:w].rearrange(
                            'p (kk j) -> p kk j', kk=kk)
                        op2 = acc_write(uu, scr, kk, jx)
                        op2.then_inc(scr_sem, 1)
                        jadd += 1

        # -------------------------------------------- gpsimd: allgather chain
        @block.gpsimd
        def _(g):
            for rep in range(reps):
                for u in range(1, S):
                    gs = rep * (S - 1) + u
                    g.wait_ge(act_sem, rep * S + u)
                    g.dma_start(out=ag_in[:, :], in_=sb_a[:, :]).then_inc(agd_sem, 16)
                    g.wait_ge(agd_sem, 32 * gs - 16)
                    g.collective_compute(
                        'AllGather', mybir.AluOpType.bypass,
                        replica_groups=[list(range(NC_COUNT))],
                        ins=[ag_in.ap().opt()], outs=[ag_out.ap().opt()],
                    ).then_inc(cc_sem, 1)
                    g.wait_ge(cc_sem, gs)
                    g.dma_start(out=sb_ag[:, :], in_=ag_out[:, :]).then_inc(agd_sem, 16)
                    g.wait_ge(agd_sem, 32 * gs)

    return nc, c

# --------------------------------------------------------------------------
# Host preprocessing
# --------------------------------------------------------------------------
def preprocess(inputs, cfg):
    c = derive(cfg)
    n, b, S = c['n'], c['b'], c['steps']
    jp, jpad, nfb, nfb0 = c['jp'], c['jpad'], c['nfb'], c['nfb0']
    e_in = c['e_in']

    x0 = np.asarray(inputs['input_data'], np.float32)         # [B, IN]
    fr = np.asarray(inputs['from_idx'], np.int64)
    to = np.asarray(inputs['to_idx'], np.int64)
    dl = np.asarray(inputs['delays'], np.int64)
    w = np.asarray(inputs['connection_weights'], np.float32)

    keep = dl < S
    fr, to, dl, w = fr[keep], to[keep], dl[keep], w[keep]
    # delay-0 edges from f >= e_in contribute 0 forever (a_0 is 0 there)
    keep0 = ~((dl == 0) & (fr >= e_in))
    fr, to, dl, w = fr[keep0], to[keep0], dl[keep0], w[keep0]

    core = to // jp
    jl = to - core * jp
    frow = fr + (jpad - jp) * (fr // jp)      # padded from-row (128-aligned blocks)

    in_maps = [dict() for _ in range(NC_COUNT)]
    for cc in range(NC_COUNT):
        for d in range(S):
            m = (core == cc) & (dl == d)
            if d == 0:
                rows = fr[m]                   # < e_in, no padding shift there
                plane = np.zeros(128 * nfb0 * jp, np.float32)
                np.add.at(plane, (rows % 128) * (nfb0 * jp) +
                          (rows // 128) * jp + jl[m], w[m] * WSCALE * SSCALE)
                in_maps[cc]['w0'] = plane.reshape(128, nfb0 * jp).astype(
                    ml_dtypes.bfloat16)
            else:
                plane = np.zeros(128 * nfb * jp, np.float32)
                np.add.at(plane, (frow[m] % 128) * (nfb * jp) +
                          (frow[m] // 128) * jp + jl[m], w[m] * WSCALE)
                in_maps[cc][f'w{d}'] = plane.reshape(128, nfb * jp).astype(
                    ml_dtypes.float8_e4m3).view(np.uint8)

    a0 = np.zeros((128, nfb0, b), np.float32)
    for fb in range(nfb0):
        lo, hi = fb * 128, min((fb + 1) * 128, e_in)
        if hi > lo:
            a0[0:hi - lo, fb, :] = x0[:, lo:hi].T
    ident = np.eye(32, dtype=ml_dtypes.bfloat16)
    for cc in range(NC_COUNT):
        in_maps[cc]['a0'] = a0.reshape(128, nfb0 * b).astype(ml_dtypes.bfloat16)
        in_maps[cc]['ident'] = ident
    return in_maps


# --------------------------------------------------------------------------
# PJRT runner (self-contained)
# --------------------------------------------------------------------------
class Runner:
    def __init__(self, nc, n_cores=NC_COUNT):
        import jax
        from jax.sharding import Mesh, PartitionSpec
        from jax.experimental.shard_map import shard_map
        import concourse.mybir as mybir
        from concourse.bass2jax import (_bass_exec_p, install_neuronx_cc_hook,
                                        partition_id_tensor)
        install_neuronx_cc_hook()
        self.jax = jax
        self.n_cores = n_cores
        partition_name = nc.partition_id_tensor.name if nc.partition_id_tensor else None
        dbg_name = nc.dbg_addr.name if nc.dbg_addr is not None else None
        in_names, out_names, out_avals, zero_outs = [], [], [], []
        for alloc in nc.m.functions[0].allocations:
            if not isinstance(alloc, mybir.MemoryLocationSet):
                continue
            name = alloc.memorylocations[0].name
            if alloc.kind == 'ExternalInput':
                if name not in (partition_name, dbg_name):
                    in_names.append(name)
            elif alloc.kind == 'ExternalOutput':
                out_names.append(name)
                shape = tuple(alloc.tensor_shape)
                dtype = mybir.dt.np(alloc.dtype)
                out_avals.append(jax.core.ShapedArray(shape, dtype))
                zero_outs.append(np.zeros(shape, dtype))
        self.in_names, self.out_names = in_names, out_names
        self.out_avals, self.zero_outs = out_avals, zero_outs
        all_in = list(in_names) + list(out_names)
        if dbg_name is not None:
            all_in.append(dbg_name)
        if partition_name is not None:
            all_in.append(partition_name)
        has_dbg = dbg_name is not None

        def _body(*args):
            operands = list(args)
            if has_dbg:
                operands.append(jax.numpy.zeros((1, 2), jax.numpy.uint32))
            if partition_name is not None:
                operands.append(partition_id_tensor())
            return tuple(_bass_exec_p.bind(
                *operands, out_avals=tuple(out_avals), in_names=tuple(all_in),
                out_names=tuple(out_names), lowering_input_output_aliases=(),
                sim_require_finite=False, sim_require_nnan=False, nc=nc))

        devices = jax.devices()[:n_cores]
        mesh = Mesh(np.asarray(devices), ('core',))
        self._fn = jax.jit(
            shard_map(_body, mesh=mesh,
                      in_specs=(PartitionSpec('core'),) * (len(in_names) + len(out_names)),
                      out_specs=(PartitionSpec('core'),) * len(out_names),
                      check_rep=False),
            keep_unused=True)
        self._sharding = jax.sharding.NamedSharding(mesh, PartitionSpec('core'))

    def put_inputs(self, in_maps):
        jax = self.jax
        dev_in = [jax.device_put(
            np.concatenate([np.asarray(m[name]) for m in in_maps], axis=0),
            self._sharding) for name in self.in_names]
        dev_zero = [jax.device_put(
            np.zeros((self.n_cores * z.shape[0], *z.shape[1:]), z.dtype),
            self._sharding) for z in self.zero_outs]
        return dev_in, dev_zero

    def run(self, dev_in, dev_zero):
        outs = self._fn(*dev_in, *dev_zero)
        self.jax.block_until_ready(outs)
        return outs

    def results(self, outs):
        return [
            {name: np.asarray(outs[i]).reshape(self.n_cores, *self.out_avals[i].shape)[c]
             for i, name in enumerate(self.out_names)}
            for c in range(self.n_cores)
        ]


# --------------------------------------------------------------------------
# public entry point
# --------------------------------------------------------------------------
_CACHE = {}


def _get_runner(cfg_key):
    if cfg_key not in _CACHE:
        cfg = dict(FULL_CFG)
        nc, c = build_bass(cfg)
        _CACHE[cfg_key] = (Runner(nc), c)
    return _CACHE[cfg_key]


def kernel(input_data, from_idx, to_idx, delays, connection_weights, steps):
    assert int(steps) == FULL_CFG['steps']
    runner, c = _get_runner('full')
    in_maps = preprocess(
        dict(input_data=input_data, from_idx=from_idx, to_idx=to_idx,
             delays=delays, connection_weights=connection_weights), FULL_CFG)
    dev_in, dev_zero = runner.put_inputs(in_maps)
    outs = runner.run(dev_in, dev_zero)
    res = runner.results(outs)
    # a_8[-e_in:] lives in core 7's trailing e_in columns == its 'a8' output
    return res[NC_COUNT - 1]['a8'].astype(np.float32)


# revision 26
# speedup vs baseline: 1.0146x; 1.0146x over previous
"""Trainium2 Bass kernel for nn_Brain (gnn_message_passing, N=20000, E=20M, B=4, S=8).

Math (faithful to the reference):
    a_0 = zeros(N); a_0[:1000] = x0
    total_u[j] = c0[j] + sum_{d=1..u-1} sum_{e in E_d} w_e * a_{u-d}[from_e]   (to_e = j)
    c0[j]      = sum_{delay-0 edges} w_e * a_0[from_e]     (constant across steps)
    a_u = tanh(total_u), u = 1..8;  output = a_8[-1000:]   (delays >= 8 never fire)

v4: fp8 DoubleRow matmuls (the v2 PE bottleneck was ~2.7ms at 1 col/cycle;
DoubleRow contracts a 256-row superblock per pass).  Both operands must be
fp8: planes stay fp8e4m3 (x WSCALE), snapshots are stored fp8e4m3 x SSCALE.
w0/a0 move to bf16 (w0 host-scaled by WSCALE*SSCALE); tanh scale =
1/(WSCALE*SSCALE).

DoubleRow ISA restrictions (walrus s3_lw_dual_fp8_restrictions):
  - matmul PSUM dst must start at partition 0 (no column tiling), and
  - stationary free-dim steps beyond the innermost must be 16B-aligned.
So the v2 rotating 32-partition PSUM regions are gone.  Instead each
interval k shares ONE accumulation region at partitions 0..4*(S-k):
row-block (u-k-1)*b holds the merged contribution to step u from all of
this interval's reads.  Row alignment comes from the stationary: the fp8
snapshot table has, per (superblock, ktile), a 20-zero prefix + 28 snapshot
slots (sp=48, so the ktile step 48 is 16B-aligned), and a read with
snapshot range slo..shi and row offset o = slo+d-k-1 loads sp[20+(slo-1-o)*4
: 20+shi*4], contracting zeros into rows below its targets.  At interval
end the DVE stages rows 0..nu_k to a rotating bf16 SBUF buffer and releases
PSUM (PE stall ~2us/interval); totals and the per-(k,u) future shifts
(scalar-queue partition-shift DMAs -> scratch -> acc_u adds) read the
staged copy.  c0 lives at PSUM partitions 32-35 (plain bf16 matmul, legal).

Read schedule (gate = highest snapshot used; gate<k reads run before the
step-k AllGather's transposes and absorb the collective latency):
  k=1:                  d1[s1]
  k=2: d7[s1](g1,restr) d2[s1..2] d1[s2]
  k=3: d6[s1..2](g2)    d3[s1..3] d1[s3]
  k=4: d5[s1..3](g3)    d4[s1..4] d2[s3..4] d1[s4]
  k=5:                  d1[s5]
  k=6: d3[s4..5](g5)    d2[s5..6] d1[s6]
  k=7:                  d1[s7](restr)
Within an interval the first writer of each PSUM bank must cover all rows
consumed from that bank (start=True coverage; validated statically).
"""
import sys
sys.path.insert(0, '/opt/trn_rl_repo')
import numpy as np
import ml_dtypes

NC_COUNT = 8
WSCALE = 64.0
SSCALE = 8.0
ZPFX = 20                                   # zero-prefix slots in sp dim
SPW = 48                                    # sp dim width (20 zeros + 28)

FULL_CFG = dict(n=20000, e_in=1000, b=4, steps=8, nbank=5, chunk_fb=8, nbuf=4,
                nrot=2, w1f=0)


def derive(cfg):
    c = dict(cfg)
    n, b, s = c['n'], c['b'], c['steps']
    jp = n // NC_COUNT                      # to-neurons per core (2500)
    jpad = ((jp + 127) // 128) * 128        # 2560
    c.update(
        jp=jp, jpad=jpad,
        lfb=jpad // 128,                    # local from-blocks per core (20)
        nfb=NC_COUNT * (jpad // 128),       # global from-blocks (160)
        fpad=NC_COUNT * jpad,               # padded from-rows (20480)
        nfb0=(c['e_in'] + 127) // 128,      # delay-0 from-blocks (8)
        bank_j=jp // c['nbank'],            # 500
        ntr=jpad // 128,                    # post-gather transpose chunks (20)
    )
    assert jp % c['nbank'] == 0 and c['bank_j'] <= 512
    assert c['chunk_fb'] % 2 == 0 and c['nfb'] % 2 == 0
    assert (s - 1) * c['b'] + ZPFX <= SPW and SPW % 16 == 0
    # ring chunks for d1 start at w1f; chunks_of handles a (necessarily even)
    # remainder chunk, which DoubleRow pairing requires
    assert c['w1f'] % 2 == 0 and 0 <= c['w1f'] < c['nfb']
    return c


def _mybir():
    import concourse.mybir as mybir
    return mybir


def make_reads(S, b):
    """Read schedule: list of dicts in PE issue order.

    gate = highest snapshot the read uses (PE gates on its copies only).
    o    = row-block offset: snapshot s lands at PSUM rows (s+d-k-1)*b.
    span = o*b + b*ns rows written (zero-prefix rows below the targets).
    """
    reads = [dict(d=0, slo=0, shi=0, itv=0, must=True, restr=False, gate=0,
                  o=0, span=b)]
    TB = {
        1: [(1, 1, 1)],
        2: [(7, 1, 1), (2, 1, 2), (1, 2, 2)],
        3: [(6, 1, 2), (3, 1, 3), (1, 3, 3)],
        4: [(5, 1, 3), (4, 1, 4), (2, 3, 4), (1, 4, 4)],
        5: [(1, 5, 5)],
        6: [(3, 4, 5), (2, 5, 6), (1, 6, 6)],
        7: [(1, 7, 7)],
    }
    for k in range(1, S):
        for d, slo, shi in TB[k]:
            assert shi <= k and slo + d >= k + 1 and shi + d <= S
            o = slo + d - k - 1
            reads.append(dict(d=d, slo=slo, shi=shi, itv=k, gate=shi,
                              must=(slo + d == k + 1), o=o,
                              span=(o + shi - slo + 1) * b,
                              restr=(slo + d == S and shi + d == S)))
    # early (gate<k) reads must precede gated ones (PE emission order is the
    # list order and the threads assume it)
    for k in range(1, S):
        kr = [r for r in reads if r['itv'] == k]
        assert kr == sorted(kr, key=lambda r: r['gate'] >= k)
        # spans must be non-increasing so each bank's first writer covers
        # every row consumed from that bank this interval
        spans = [r['span'] for r in kr]
        assert spans == sorted(spans, reverse=True)
    # coverage: every (d, u) pair exactly once, on time
    seen = set()
    for r in reads[1:]:
        for s in range(r['slo'], r['shi'] + 1):
            key = (r['d'], s + r['d'])
            assert key not in seen and s + r['d'] <= S
            assert s + r['d'] >= r['itv'] + 1
            seen.add(key)
    assert seen == {(d, u) for d in range(1, S) for u in range(d + 1, S + 1)}
    return reads


def interval_info(reads, S, b, nbank, nbk_r):
    """Per-interval: nu (rows staged), consumed (u -> (row0, restr_src)),
    and per-read per-bank start/stop flags."""
    info = {}
    for k in range(1, S):
        kr = [(ri, r) for ri, r in enumerate(reads) if r['itv'] == k]
        nu = max(r['span'] for _, r in kr)
        consumed = {}
        for _, r in kr:
            for i in range(r['shi'] - r['slo'] + 1):
                u = r['slo'] + r['d'] + i
                ro = (r['o'] + i) * b
                prev = consumed.get(u)
                if prev is not None:
                    assert prev == (ro, r['restr']), 'mixed src windows'
                consumed[u] = (ro, r['restr'])
        # per-bank first/last writer -> start/stop flags per read
        first_w, last_w = {}, {}
        for ri, r in kr:
            banks = range(nbk_r) if r['restr'] else range(nbank)
            for bi in banks:
                first_w.setdefault(bi, ri)
                last_w[bi] = ri
        starts, stops = {}, {}
        for ri, r in kr:
            banks = range(nbk_r) if r['restr'] else range(nbank)
            starts[ri] = {bi: (first_w[bi] == ri) for bi in banks}
            stops[ri] = {bi: (last_w[bi] == ri) for bi in banks}
            # coverage check: first writer of each bank spans all consumed
            # rows of that bank this interval
            for bi in banks:
                if first_w[bi] != ri:
                    continue
                for u, (ro, rsrc) in consumed.items():
                    src_banks = range(nbk_r) if rsrc else range(nbank)
                    if bi in src_banks or u == k + 1:
                        assert ro + b <= r['span'], (k, u, bi)
        info[k] = dict(nu=nu, consumed=consumed, starts=starts, stops=stops)
    return info


# --------------------------------------------------------------------------
# Bass program
# --------------------------------------------------------------------------
def build_bass(cfg, reps=1):
    from concourse import bass
    mybir = _mybir()
    c = derive(cfg)
    n, b, S = c['n'], c['b'], c['steps']
    jp, jpad, lfb, nfb, nfb0 = c['jp'], c['jpad'], c['lfb'], c['nfb'], c['nfb0']
    nbank, bank_j, chunk_fb = c['nbank'], c['bank_j'], c['chunk_fb']
    NBUF = c['nbuf']
    NROT = c['nrot']
    ntr = c['ntr']
    NSNAP = S - 1
    TGRP = 16
    ngrp = (ntr + TGRP - 1) // TGRP
    e_in = c['e_in']
    jr = jp - e_in                          # restricted col start (1500)
    rbank0 = jr // bank_j                   # first bank of restricted cols (3)
    nbk_r = e_in // bank_j                  # banks in restricted reads (2)
    assert rbank0 * bank_j == jr

    reads = make_reads(S, b)
    NREADS = len(reads)                     # 17
    iinfo = interval_info(reads, S, b, nbank, nbk_r)

    rhs_elems = chunk_fb * jp               # ring buf bytes per partition (20000)
    w0_ch = chunk_fb // 2                   # bf16 fb per chunk (4)
    W1F = c['w1f']                          # d1 from-blocks resident in SBUF

    def chunks_of(lo, total, ch):
        out, x = [], lo
        while x < total:
            out.append((x, min(ch, total - x)))
            x += ch
        return out

    chunk_list = []
    cum_end = []                            # per-read end chunk index (per rep)
    for ri, r in enumerate(reads):
        if r['d'] == 0:
            ch_list = chunks_of(0, nfb0, w0_ch)
        elif r['d'] == 1:
            # first W1F blocks come from the resident SBUF copy, not the ring
            ch_list = chunks_of(W1F, nfb, chunk_fb)
        else:
            ch_list = chunks_of(0, nfb, chunk_fb)
        for (f0, ch) in ch_list:
            chunk_list.append((ri, f0, ch))
        cum_end.append(len(chunk_list))
    NCHUNK = len(chunk_list)

    # shift plan: per (k, u>k+1): one partition-shift DMA + one acc add.
    # acc_us: accumulator targets.
    shift_plan = []                         # (k, u, ro, restr_src)
    for k in range(1, S):
        for u in sorted(iinfo[k]['consumed']):
            if u == k + 1:
                continue
            ro, rsrc = iinfo[k]['consumed'][u]
            shift_plan.append((k, u, ro, rsrc))
    n_shifts = len(shift_plan)
    acc_us = sorted({u for (_, u, _, _) in shift_plan})
    # per-u first-shift index (to pick copy vs add into acc) per rep
    first_shift_of_u = {}
    for jx, (k, u, _, _) in enumerate(shift_plan):
        first_shift_of_u.setdefault(u, jx)
    # tanh gating: tot_sem incs are c0-copy (u=1) then one per interval total
    # -> tanh(u) waits tot_sem >= rep*S + u.

    nc = bass.Bass(target_bir_lowering=False)

    planes = [nc.declare_dram_parameter(f'w{d}', [128, nfb * jp], mybir.dt.uint8,
                                        isOutput=False) for d in range(1, S)]
    w0_t = nc.declare_dram_parameter('w0', [128, nfb0 * jp], mybir.dt.bfloat16,
                                     isOutput=False)
    a0_t = nc.declare_dram_parameter('a0', [128, nfb0 * b], mybir.dt.bfloat16,
                                     isOutput=False)
    id_t = nc.declare_dram_parameter('ident', [32, 32], mybir.dt.bfloat16,
                                     isOutput=False)
    out_t = nc.declare_dram_parameter('a8', [b, e_in], mybir.dt.float32,
                                      isOutput=True)
    ag_in = nc.dram_tensor('ag_in', [b, jpad], mybir.dt.bfloat16)
    ag_out = nc.dram_tensor('ag_out', [NC_COUNT * b, jpad], mybir.dt.bfloat16)

    from contextlib import ExitStack
    with ExitStack() as _es:
        init_sem = _es.enter_context(nc.semaphore('init_sem'))
        w1c_sem = _es.enter_context(nc.semaphore('w1c_sem'))
        pln_sems = [_es.enter_context(nc.semaphore(f'pln{i}')) for i in range(NBUF)]
        free_sem = _es.enter_context(nc.semaphore('free_sem'))
        drain_sem = _es.enter_context(nc.semaphore('drain_sem'))
        tot_sem = _es.enter_context(nc.semaphore('tot_sem'))
        act_sem = _es.enter_context(nc.semaphore('act_sem'))
        agd_sem = _es.enter_context(nc.semaphore('agd_sem'))
        cc_sem = _es.enter_context(nc.semaphore('cc_sem'))
        tr_sem = _es.enter_context(nc.semaphore('tr_sem'))
        cp_sem = _es.enter_context(nc.semaphore('cp_sem'))
        ms_sem = _es.enter_context(nc.semaphore('ms_sem'))
        fin_sem = _es.enter_context(nc.semaphore('fin_sem'))
        shf_sem = _es.enter_context(nc.semaphore('shf_sem'))
        scr_sem = _es.enter_context(nc.semaphore('scr_sem'))
        sb_rhs = [_es.enter_context(nc.sbuf_tensor(f'sb_rhs{i}', [128, rhs_elems], mybir.dt.uint8))
                  for i in range(NBUF)]
        sb_w1c = (_es.enter_context(nc.sbuf_tensor('sb_w1c', [128, W1F * jp], mybir.dt.uint8))
                  if W1F else None)
        sb_snap = _es.enter_context(nc.sbuf_tensor('sb_snap', [128, nfb * SPW], mybir.dt.float8e4))
        sb_a0 = _es.enter_context(nc.sbuf_tensor('sb_a0', [128, nfb0 * b], mybir.dt.bfloat16))
        sb_id = _es.enter_context(nc.sbuf_tensor('sb_id', [32, 32], mybir.dt.bfloat16))
        sb_c0 = _es.enter_context(nc.sbuf_tensor('sb_c0', [b, jp], mybir.dt.float32))
        sb_tot = _es.enter_context(nc.sbuf_tensor('sb_tot', [b, jp], mybir.dt.float32))
        sb_tmp = _es.enter_context(nc.sbuf_tensor('sb_tmp', [b, jp], mybir.dt.float32))
        sb_tot8 = _es.enter_context(nc.sbuf_tensor('sb_tot8', [b, e_in], mybir.dt.float32))
        sb_a = _es.enter_context(nc.sbuf_tensor('sb_a', [b, jpad], mybir.dt.bfloat16))
        sb_a32 = _es.enter_context(nc.sbuf_tensor('sb_a32', [b, e_in], mybir.dt.float32))
        sb_ag = _es.enter_context(nc.sbuf_tensor('sb_ag', [NC_COUNT * b, jpad], mybir.dt.bfloat16))
        sb_acc = {u: _es.enter_context(nc.sbuf_tensor(
            f'sb_acc{u}', [b, e_in if u == S else jp], mybir.dt.float32))
            for u in acc_us}
        # staged PSUM rows (bf16, rotating): totals + shifts read from here
        sb_stg = _es.enter_context(nc.sbuf_tensor('sb_stg', [128, NROT * jp], mybir.dt.bfloat16))
        sb_scr = _es.enter_context(nc.sbuf_tensor('sb_scr', [b, jp], mybir.dt.bfloat16))
        ps = _es.enter_context(nc.psum_tensor('ps', [128, nbank * 512], mybir.dt.float32))
        psT = _es.enter_context(nc.psum_tensor('psT', [128, 2 * TGRP * NC_COUNT * b], mybir.dt.bfloat16))

        block = _es.enter_context(nc.Block())
        AFT = mybir.ActivationFunctionType
        DR = mybir.MatmulPerfMode.DoubleRow
        psr = ps.ap().rearrange('p (k j) -> p k j', k=nbank)
        # snapshot table: [p, superblock f2, ktile t, sp] with 20-zero prefix
        snapv = sb_snap.ap().rearrange('p (f2 t sp) -> p f2 t sp',
                                       f2=nfb // 2, t=2)
        a03 = sb_a0.ap().rearrange('p (f b) -> p f b', f=nfb0)
        psT4 = psT.ap().rearrange('p (h t cb) -> p h t cb', h=2, t=TGRP)
        sb_ag3 = sb_ag.ap().rearrange('cb (k p) -> cb k p', p=128)
        c0_3 = sb_c0.ap().rearrange('p (k j) -> p k j', k=nbank)
        tot_3 = sb_tot.ap().rearrange('p (k j) -> p k j', k=nbank)
        tmp_3 = sb_tmp.ap().rearrange('p (k j) -> p k j', k=nbank)
        tot8_3 = sb_tot8.ap().rearrange('p (k j) -> p k j', k=nbk_r)
        stg4 = sb_stg.ap().rearrange('p (r k j) -> p r k j', r=NROT, k=nbank)

        def acc_3(u):
            k = nbk_r if u == S else nbank
            return sb_acc[u].ap().rearrange('p (k j) -> p k j', k=k)

        def rot_of(k_abs):
            return k_abs % NROT

        # -------------------------------------------- sync: plane DMA stream
        @block.sync
        def _(s):
            s.dma_start(out=sb_a0[:, :], in_=a0_t[:, :]).then_inc(init_sem, 16)
            s.dma_start(out=sb_id[:, :], in_=id_t[:, :]).then_inc(init_sem, 16)
            if W1F:
                # one-time fill of the resident d1-plane prefix (reused by
                # every d1 read of every rep)
                s.dma_start(out=sb_w1c[:, :],
                            in_=planes[0].ap()[:, 0:W1F * jp]).then_inc(w1c_sem, 16)
            gi = 0
            for rep in range(reps):
                for (ri, f0, ch) in chunk_list:
                    r = reads[ri]
                    if gi >= NBUF:
                        s.wait_ge(free_sem, gi - NBUF + 1)
                    buf = sb_rhs[gi % NBUF]
                    if r['d'] == 0:
                        src = w0_t.ap().rearrange('p (f j) -> p f j', f=nfb0)[
                            :, f0:f0 + ch, :]
                        dst = buf.ap().bitcast(mybir.dt.bfloat16)[
                            :, 0:ch * jp].rearrange('p (c j) -> p c j', c=ch)
                    elif r['restr']:
                        src = planes[r['d'] - 1].ap().rearrange(
                            'p (f j) -> p f j', f=nfb)[:, f0:f0 + ch, jr:jp]
                        dst = buf[:, 0:ch * e_in].rearrange('p (c j) -> p c j', c=ch)
                    else:
                        src = planes[r['d'] - 1].ap().rearrange(
                            'p (f j) -> p f j', f=nfb)[:, f0:f0 + ch, :]
                        dst = buf[:, 0:ch * jp].rearrange('p (c j) -> p c j', c=ch)
                    s.dma_start(out=dst, in_=src).then_inc(pln_sems[gi % NBUF], 16)
                    gi += 1
                s.wait_ge(act_sem, rep * S + S)
                s.dma_start(out=out_t[:, :], in_=sb_a32[:, :]).then_inc(fin_sem, 16)
            s.wait_ge(fin_sem, 16 * reps)

        # -------------------------------------------- tensor: matmuls + transposes
        @block.tensor
        def _(t):
            ci = 0
            gtr = 0

            def emit_read(ri, rep, k):
                nonlocal ci
                r = reads[ri]
                d, slo, shi, o = r['d'], r['slo'], r['shi'], r['o']
                nbk = nbk_r if r['restr'] else nbank
                jw = e_in if r['restr'] else jp
                span = r['span']
                if r['gate'] >= 1:
                    t.wait_ge(cp_sem, (rep * (S - 1) + r['gate']) * ngrp)
                starts = iinfo[k]['starts'][ri] if k >= 1 else None
                stops = iinfo[k]['stops'][ri] if k >= 1 else None
                cached = (d == 1 and W1F > 0)
                if cached:
                    # resident d1-plane prefix: matmuls straight out of SBUF
                    sp0 = ZPFX + (slo - 1 - o) * b
                    rhs4c = sb_w1c.ap().bitcast(mybir.dt.float8e4).rearrange(
                        'p (c2 t j) -> p c2 t j', c2=W1F // 2, t=2)
                    jb = jr if r['restr'] else 0
                    t.wait_ge(w1c_sem, 16)
                    if r['gate'] >= 1:
                        pass  # cp wait already emitted above
                    for cc2 in range(W1F // 2):
                        lhsT = snapv[:, cc2, :, sp0:sp0 + span]
                        for bi in range(nbk):
                            mm = t.matmul(
                                psr[0:span, bi, 0:bank_j],
                                lhsT,
                                rhs4c[:, cc2, :,
                                      jb + bi * bank_j:jb + (bi + 1) * bank_j],
                                start=(starts[bi] and cc2 == 0),
                                stop=False,
                                perf_mode=DR,
                                skip_group_check=True)
                first_c, last_c = (cum_end[ri - 1] if ri else 0), cum_end[ri]
                for cj in range(first_c, last_c):
                    _, f0, ch = chunk_list[cj]
                    if ci == 0:
                        t.wait_ge(init_sem, 32)
                    t.wait_ge(pln_sems[ci % NBUF], 16 * (ci // NBUF + 1))
                    buf = sb_rhs[ci % NBUF]
                    if d == 0:
                        rhs3 = buf.ap().bitcast(mybir.dt.bfloat16)[
                            :, 0:ch * jw].rearrange('p (c j) -> p c j', c=ch)
                        for cc in range(ch):
                            lhsT = a03[:, f0 + cc, :]
                            for bi in range(nbk):
                                mm = t.matmul(
                                    psr[32:32 + b, bi, 0:bank_j],
                                    lhsT,
                                    rhs3[:, cc, bi * bank_j:(bi + 1) * bank_j],
                                    start=(cj == first_c and cc == 0),
                                    stop=(cj == last_c - 1 and cc == ch - 1),
                                    skip_group_check=True)
                    else:
                        # DoubleRow: [p, 2, jw] moving vs [p, 2, sp-slice] fp8
                        rhs4 = buf.ap().bitcast(mybir.dt.float8e4)[
                            :, 0:ch * jw].rearrange('p (c2 t j) -> p c2 t j',
                                                    c2=ch // 2, t=2)
                        sp0 = ZPFX + (slo - 1 - o) * b
                        for cc2 in range(ch // 2):
                            f2 = (f0 + 2 * cc2) // 2
                            lhsT = snapv[:, f2, :, sp0:sp0 + span]
                            for bi in range(nbk):
                                mm = t.matmul(
                                    psr[0:span, bi, 0:bank_j],
                                    lhsT,
                                    rhs4[:, cc2, :, bi * bank_j:(bi + 1) * bank_j],
                                    start=(starts[bi] and cj == first_c
                                           and cc2 == 0 and not cached),
                                    stop=(stops[bi] and cj == last_c - 1
                                          and cc2 == ch // 2 - 1),
                                    perf_mode=DR,
                                    skip_group_check=True)
                    mm.then_inc(free_sem, 1)
                    ci += 1

            for rep in range(reps):
                for k in range(0, S):
                    if k >= 1:
                        # region reuse: stages of intervals 1..k-1 (this rep)
                        # and all of the previous rep must be done
                        t.wait_ge(drain_sem, rep * (S - 1) + k - 1)
                    else:
                        if rep > 0:
                            # c0 PSUM region (parts 32+) reuse: previous rep's
                            # c0 copy must be done
                            t.wait_ge(tot_sem, (rep - 1) * S + 1)
                    for ri, r in enumerate(reads):
                        if r['itv'] == k and r['gate'] < k:
                            emit_read(ri, rep, k)
                    if 1 <= k <= S - 1:
                        gs = rep * (S - 1) + k
                        t.wait_ge(agd_sem, 32 * gs)
                        for g in range(ngrp):
                            if gtr >= 2:
                                t.wait_ge(cp_sem, gtr - 1)
                            half = gtr % 2
                            k0 = g * TGRP
                            kcnt = min(TGRP, ntr - k0)
                            for kk in range(kcnt):
                                mm = t.transpose(psT4[:, half, kk, :],
                                                 sb_ag3[:, k0 + kk, :], sb_id[:, :])
                            mm.then_inc(tr_sem, 1)
                            gtr += 1
                    for ri, r in enumerate(reads):
                        if r['itv'] == k and r['gate'] >= k:
                            emit_read(ri, rep, k)

        # -------------------------------------------- scalar: tanh + shift DMAs
        @block.scalar
        def _(a):
            a.wait_ge(ms_sem, 2)
            jshift = 0
            for rep in range(reps):
                for u in range(1, S + 1):
                    a.wait_ge(tot_sem, rep * S + u)
                    if u == 1:
                        a.activation(sb_a[:, 0:jp], sb_c0[:, :], AFT.Tanh,
                                     scale=1.0 / (WSCALE * SSCALE)).then_inc(act_sem, 1)
                    elif u < S:
                        a.activation(sb_a[:, 0:jp], sb_tot[:, :], AFT.Tanh,
                                     scale=1.0 / (WSCALE * SSCALE)).then_inc(act_sem, 1)
                    else:
                        if rep > 0:
                            a.wait_ge(fin_sem, 16 * rep)
                        a.activation(sb_a32[:, :], sb_tot8[:, :], AFT.Tanh,
                                     scale=1.0 / (WSCALE * SSCALE)).then_inc(act_sem, 1)
                    # interval k=u-1's future shifts: stg -> scratch
                    if u <= S - 1:
                        k = u - 1
                        for (kk2, uu, ro, rsrc) in shift_plan:
                            if kk2 != k:
                                continue
                            # stage(k) done is implied by tot_sem(u) above
                            if jshift >= 1:
                                a.wait_ge(scr_sem, jshift)
                            nbs = nbk_r if uu == S else nbank
                            blo = 0 if (uu < S or rsrc) else rbank0
                            w = nbs * bank_j
                            rr = rot_of(rep * (S - 1) + k - 1)
                            a.dma_start(
                                out=sb_scr.ap()[0:b, 0:w],
                                in_=stg4[ro:ro + b, rr, blo:blo + nbs, :],
                            ).then_inc(shf_sem, 16)
                            jshift += 1

        # -------------------------------------------- vector: stage + totals + snap copies
        @block.vector
        def _(v):
            gcp = 0
            jadd = 0
            cls_last = {}
            v.memset(sb_a[:, :], 0.0).then_inc(ms_sem, 1)
            v.memset(sb_snap[:, :], 0.0).then_inc(ms_sem, 1)
            for rep in range(reps):

                def acc_write(u, src, kk, jx):
                    if jx == first_shift_of_u[u]:
                        return v.tensor_copy(acc_3(u)[:, 0:kk, :], src)
                    v.tensor_tensor(tmp_3[:, 0:kk, :],
                                    acc_3(u)[:, 0:kk, :], src,
                                    mybir.AluOpType.add)
                    return v.tensor_copy(acc_3(u)[:, 0:kk, :],
                                         tmp_3[:, 0:kk, :])

                # w0 -> c0 (PSUM partitions 32..32+b)
                v.wait_ge(free_sem, rep * NCHUNK + cum_end[0])
                v.tensor_copy(c0_3, psr[32:32 + b, 0:nbank, 0:bank_j]
                              ).then_inc(tot_sem, 1)

                for k in range(1, S):
                    # snapshot-k copies (scaled bf16 -> fp8)
                    for g in range(ngrp):
                        v.wait_ge(tr_sem, gcp + 1)
                        half = gcp % 2
                        k0 = g * TGRP
                        kcnt = min(TGRP, ntr - k0)
                        src = psT4[:, half, 0:kcnt, :].rearrange(
                            'p t (c b) -> p t c b', c=NC_COUNT)
                        dst = sb_snap.ap().rearrange(
                            'p (c kl sp) -> p kl c sp', c=NC_COUNT, sp=SPW)[
                            :, k0:k0 + kcnt, :,
                            ZPFX + (k - 1) * b:ZPFX + k * b]
                        v.tensor_scalar_mul(dst, src, SSCALE).then_inc(cp_sem, 1)
                        gcp += 1

                    u = k + 1
                    nu = iinfo[k]['nu']
                    last_ri = max(ri for ri, r in enumerate(reads)
                                  if r['itv'] == k)
                    v.wait_ge(free_sem, rep * NCHUNK + cum_end[last_ri])
                    # stage rows 0..nu to the rotating bf16 buffer, release PSUM
                    rr = rot_of(rep * (S - 1) + k - 1)
                    if rr in cls_last:
                        v.wait_ge(shf_sem, 16 * cls_last[rr])
                    v.tensor_copy(stg4[0:nu, rr, :, :],
                                  psr[0:nu, 0:nbank, 0:bank_j]
                                  ).then_inc(drain_sem, 1)
                    lastj = [jx for jx, (kk2, _, _, _) in enumerate(shift_plan)
                             if kk2 == k]
                    if lastj:
                        cls_last[rr] = rep * n_shifts + lastj[-1] + 1
                    # total_u = c0 + stg rows 0..b + acc_u
                    direct_restr = reads[last_ri]['restr'] and u == S
                    addends = [('c0', None), ('stg', rr)]
                    # every shift (k', u) has k' <= u-2 = k-1, so acc_u is
                    # complete by now whenever it exists at all
                    if u in sb_acc:
                        addends.append(('acc', u))
                    A = len(addends)
                    if u < S:
                        c0v, totv, tmpv = c0_3, tot_3, tmp_3
                        stgv = stg4[0:b, rr, 0:nbank, :]
                    else:
                        c0v = c0_3[:, rbank0:rbank0 + nbk_r, :]
                        totv = tot8_3
                        tmpv = tmp_3[:, 0:nbk_r, :]
                        blo = 0 if direct_restr else rbank0
                        stgv = stg4[0:b, rr, blo:blo + nbk_r, :]

                    def addend_ap(spec):
                        kind, x = spec
                        if kind == 'c0':
                            return c0v
                        if kind == 'stg':
                            return stgv
                        kk = nbk_r if u == S else nbank
                        return acc_3(u)[:, 0:kk, :]

                    cur = addend_ap(addends[0])
                    op = None
                    for j in range(A - 1):
                        target = totv if (A - 2 - j) % 2 == 0 else tmpv
                        op = v.tensor_tensor(target, cur, addend_ap(addends[j + 1]),
                                             mybir.AluOpType.add)
                        cur = target
                    op.then_inc(tot_sem, 1)
                    # scratch adds for interval k's shifts
                    for jx, (kk2, uu, ro, rsrc) in enumerate(shift_plan):
                        if kk2 != k:
                            continue
                        v.wait_ge(shf_sem, 16 * (jadd + 1))
                        kk = nbk_r if uu == S else nbank
                        w = e_in if uu == S else jp
                        scr = sb_scr.ap()[0:b, 0:w].rearrange(
                            'p (kk j) -> p kk j', kk=kk)
                        op2 = acc_write(uu, scr, kk, jx)
                        op2.then_inc(scr_sem, 1)
                        jadd += 1

        # -------------------------------------------- gpsimd: allgather chain
        @block.gpsimd
        def _(g):
            for rep in range(reps):
                for u in range(1, S):
                    gs = rep * (S - 1) + u
                    g.wait_ge(act_sem, rep * S + u)
                    g.dma_start(out=ag_in[:, :], in_=sb_a[:, :]).then_inc(agd_sem, 16)
                    g.wait_ge(agd_sem, 32 * gs - 16)
                    g.collective_compute(
                        'AllGather', mybir.AluOpType.bypass,
                        replica_groups=[list(range(NC_COUNT))],
                        ins=[ag_in.ap().opt()], outs=[ag_out.ap().opt()],
                    ).then_inc(cc_sem, 1)
                    g.wait_ge(cc_sem, gs)
                    g.dma_start(out=sb_ag[:, :], in_=ag_out[:, :]).then_inc(agd_sem, 16)
                    g.wait_ge(agd_sem, 32 * gs)

    return nc, c

# --------------------------------------------------------------------------
# Host preprocessing
# --------------------------------------------------------------------------
def preprocess(inputs, cfg):
    c = derive(cfg)
    n, b, S = c['n'], c['b'], c['steps']
    jp, jpad, nfb, nfb0 = c['jp'], c['jpad'], c['nfb'], c['nfb0']
    e_in = c['e_in']

    x0 = np.asarray(inputs['input_data'], np.float32)         # [B, IN]
    fr = np.asarray(inputs['from_idx'], np.int64)
    to = np.asarray(inputs['to_idx'], np.int64)
    dl = np.asarray(inputs['delays'], np.int64)
    w = np.asarray(inputs['connection_weights'], np.float32)

    keep = dl < S
    fr, to, dl, w = fr[keep], to[keep], dl[keep], w[keep]
    # delay-0 edges from f >= e_in contribute 0 forever (a_0 is 0 there)
    keep0 = ~((dl == 0) & (fr >= e_in))
    fr, to, dl, w = fr[keep0], to[keep0], dl[keep0], w[keep0]

    core = to // jp
    jl = to - core * jp
    frow = fr + (jpad - jp) * (fr // jp)      # padded from-row (128-aligned blocks)

    in_maps = [dict() for _ in range(NC_COUNT)]
    for cc in range(NC_COUNT):
        for d in range(S):
            m = (core == cc) & (dl == d)
            if d == 0:
                rows = fr[m]                   # < e_in, no padding shift there
                plane = np.zeros(128 * nfb0 * jp, np.float32)
                np.add.at(plane, (rows % 128) * (nfb0 * jp) +
                          (rows // 128) * jp + jl[m], w[m] * WSCALE * SSCALE)
                in_maps[cc]['w0'] = plane.reshape(128, nfb0 * jp).astype(
                    ml_dtypes.bfloat16)
            else:
                plane = np.zeros(128 * nfb * jp, np.float32)
                np.add.at(plane, (frow[m] % 128) * (nfb * jp) +
                          (frow[m] // 128) * jp + jl[m], w[m] * WSCALE)
                in_maps[cc][f'w{d}'] = plane.reshape(128, nfb * jp).astype(
                    ml_dtypes.float8_e4m3).view(np.uint8)

    a0 = np.zeros((128, nfb0, b), np.float32)
    for fb in range(nfb0):
        lo, hi = fb * 128, min((fb + 1) * 128, e_in)
        if hi > lo:
            a0[0:hi - lo, fb, :] = x0[:, lo:hi].T
    ident = np.eye(32, dtype=ml_dtypes.bfloat16)
    for cc in range(NC_COUNT):
        in_maps[cc]['a0'] = a0.reshape(128, nfb0 * b).astype(ml_dtypes.bfloat16)
        in_maps[cc]['ident'] = ident
    return in_maps


# --------------------------------------------------------------------------
# PJRT runner (self-contained)
# --------------------------------------------------------------------------
class Runner:
    def __init__(self, nc, n_cores=NC_COUNT):
        import jax
        from jax.sharding import Mesh, PartitionSpec
        from jax.experimental.shard_map import shard_map
        import concourse.mybir as mybir
        from concourse.bass2jax import (_bass_exec_p, install_neuronx_cc_hook,
                                        partition_id_tensor)
        install_neuronx_cc_hook()
        self.jax = jax
        self.n_cores = n_cores
        partition_name = nc.partition_id_tensor.name if nc.partition_id_tensor else None
        dbg_name = nc.dbg_addr.name if nc.dbg_addr is not None else None
        in_names, out_names, out_avals, zero_outs = [], [], [], []
        for alloc in nc.m.functions[0].allocations:
            if not isinstance(alloc, mybir.MemoryLocationSet):
                continue
            name = alloc.memorylocations[0].name
            if alloc.kind == 'ExternalInput':
                if name not in (partition_name, dbg_name):
                    in_names.append(name)
            elif alloc.kind == 'ExternalOutput':
                out_names.append(name)
                shape = tuple(alloc.tensor_shape)
                dtype = mybir.dt.np(alloc.dtype)
                out_avals.append(jax.core.ShapedArray(shape, dtype))
                zero_outs.append(np.zeros(shape, dtype))
        self.in_names, self.out_names = in_names, out_names
        self.out_avals, self.zero_outs = out_avals, zero_outs
        all_in = list(in_names) + list(out_names)
        if dbg_name is not None:
            all_in.append(dbg_name)
        if partition_name is not None:
            all_in.append(partition_name)
        has_dbg = dbg_name is not None

        def _body(*args):
            operands = list(args)
            if has_dbg:
                operands.append(jax.numpy.zeros((1, 2), jax.numpy.uint32))
            if partition_name is not None:
                operands.append(partition_id_tensor())
            return tuple(_bass_exec_p.bind(
                *operands, out_avals=tuple(out_avals), in_names=tuple(all_in),
                out_names=tuple(out_names), lowering_input_output_aliases=(),
                sim_require_finite=False, sim_require_nnan=False, nc=nc))

        devices = jax.devices()[:n_cores]
        mesh = Mesh(np.asarray(devices), ('core',))
        self._fn = jax.jit(
            shard_map(_body, mesh=mesh,
                      in_specs=(PartitionSpec('core'),) * (len(in_names) + len(out_names)),
                      out_specs=(PartitionSpec('core'),) * len(out_names),
                      check_rep=False),
            keep_unused=True)
        self._sharding = jax.sharding.NamedSharding(mesh, PartitionSpec('core'))

    def put_inputs(self, in_maps):
        jax = self.jax
        dev_in = [jax.device_put(
            np.concatenate([np.asarray(m[name]) for m in in_maps], axis=0),
            self._sharding) for name in self.in_names]
        dev_zero = [jax.device_put(
            np.zeros((self.n_cores * z.shape[0], *z.shape[1:]), z.dtype),
            self._sharding) for z in self.zero_outs]
        return dev_in, dev_zero

    def run(self, dev_in, dev_zero):
        outs = self._fn(*dev_in, *dev_zero)
        self.jax.block_until_ready(outs)
        return outs

    def results(self, outs):
        return [
            {name: np.asarray(outs[i]).reshape(self.n_cores, *self.out_avals[i].shape)[c]
             for i, name in enumerate(self.out_names)}
            for c in range(self.n_cores)
        ]


# --------------------------------------------------------------------------
# public entry point
# --------------------------------------------------------------------------
_CACHE = {}


def _get_runner(cfg_key):
    if cfg_key not in _CACHE:
        cfg = dict(FULL_CFG)
        nc, c = build_bass(cfg)
        _CACHE[cfg_key] = (Runner(nc), c)
    return _CACHE[cfg_key]


def kernel(input_data, from_idx, to_idx, delays, connection_weights, steps):
    assert int(steps) == FULL_CFG['steps']
    runner, c = _get_runner('full')
    in_maps = preprocess(
        dict(input_data=input_data, from_idx=from_idx, to_idx=to_idx,
             delays=delays, connection_weights=connection_weights), FULL_CFG)
    dev_in, dev_zero = runner.put_inputs(in_maps)
    outs = runner.run(dev_in, dev_zero)
    res = runner.results(outs)
    # a_8[-e_in:] lives in core 7's trailing e_in columns == its 'a8' output
    return res[NC_COUNT - 1]['a8'].astype(np.float32)
